# revision 8
# baseline (speedup 1.0000x reference)
"""Multi-head attention block (B=4, N=1370, C=1024, H=16) on 8 NeuronCores.

Sharding: core i -> batch i//2, head-group i%2 (8 heads = 512 channels).
Each core computes qkv^T = W_shard @ tok_b^T, per-head attention in the
transposed (S^T) layout with the softmax denominator folded into the PV
matmul as an extra ones column, then the projection partial
final^T = proj_w[:, shard].T-slice @ out^T.  Host sums the two partials
per batch and adds proj_b.

All heavy matmuls run in float32r (1 cycle/row at free-dim >= 256,
~1.6e-4 relative error).  exp(S^T) and V are bf16 (PV operands).
"""

import numpy as np

B, N, C = 4, 1370, 1024
H_PER_CORE = 8
HD = 64          # head dim
CH = 512         # channels per core
NKT = 11         # 128-key tiles (10*128 + 90)
QC = [(0, 512), (512, 512), (1024, 346)]  # query free-dim chunks (bank aligned)

_NC_CACHE = {}


def _legalize_multiwait(nc, mybir):
    """This walrus build accepts only one sync wait per instruction; Tile's
    exit drain stacks one wait per live semaphore.  Hoist extras onto no-ops
    inserted just before the offending instruction."""
    for f in nc.m.functions:
        for bb in f.blocks:
            insts = bb.instructions
            i = 0
            while i < len(insts):
                inst = insts[i]
                si = inst.sync_info
                if si is not None and len(si.on_wait) > 1:
                    waits = list(si.on_wait)
                    for j, w in enumerate(waits[:-1]):
                        nop = mybir.InstNoOp(
                            name=f"{inst.name}-waitsplit-{j}", ins=[], outs=[]
                        )
                        nop.engine = inst.engine
                        nop.sync_info = mybir.SyncInfo(on_wait=[w], on_update=[])
                        insts.insert(i, nop)
                        nc.register_instruction(nop, overwrite=True)
                        i += 1
                    inst.sync_info = mybir.SyncInfo(
                        on_wait=[waits[-1]], on_update=list(si.on_update)
                    )
                i += 1


def build_nc():
    import concourse.bass as bass
    import concourse.mybir as mybir
    import concourse.tile as tile
    from contextlib import ExitStack

    f32 = mybir.dt.float32
    f32r = mybir.dt.float32r
    bf16 = mybir.dt.bfloat16
    AF = mybir.ActivationFunctionType
    ALU = mybir.AluOpType

    nc = bass.Bass()

    tokT = nc.dram_tensor("tokT", [C, N], f32r, kind="ExternalInput")
    wqkT = nc.dram_tensor("wqkT", [C, 1024], f32r, kind="ExternalInput")
    qkb = nc.dram_tensor("qkb", [1024], f32, kind="ExternalInput")
    wvT = nc.dram_tensor("wvT", [C, CH], f32r, kind="ExternalInput")
    vbb = nc.dram_tensor("vbb", [128, CH], f32, kind="ExternalInput")
    pwT = nc.dram_tensor("pwT", [CH, 1024], f32r, kind="ExternalInput")
    onesd = nc.dram_tensor("onesd", [128, 64], f32r, kind="ExternalInput")
    outT = nc.dram_tensor("outT", [1024, N], f32, kind="ExternalOutput")

    with tile.TileContext(nc) as tc, ExitStack() as ctx:
        persist = ctx.enter_context(tc.tile_pool(name="persist", bufs=1))

        ones_t = persist.tile([128, 64], f32r)
        pwT_t = persist.tile([128, 4, 1024], f32r)
        qkT_t = persist.tile([128, 8, N], f32r)   # q ch 0-511 (mt 0-3), k ch (mt 4-7)
        v_t = persist.tile([128, NKT, 8 * 65], bf16)  # per head: 64 V cols + ones col
        outT_t = persist.tile([128, 4, N], f32r)  # attention out^T per pair
        nc.sync.dma_start(pwT_t[:], pwT.rearrange("(t p) c -> p t c", p=128))
        nc.sync.dma_start(ones_t[:], onesd[:])
        vt_heads = v_t[:].rearrange("p t (h x) -> p t h x", x=65)
        nc.vector.memset(vt_heads[:, :, :, 64:65], 1.0)

        # ---- phase 1: qkv projection (scoped pools, freed afterwards) ----
        with tc.tile_pool(name="p1", bufs=1) as p1, \
             tc.tile_pool(name="ps_qk", bufs=2, space="PSUM") as ps_qk, \
             tc.tile_pool(name="ps_v", bufs=2, space="PSUM") as ps_v:
            tokT_t = p1.tile([128, 8, N], f32r)
            wqkT_t = p1.tile([128, 8, 1024], f32r)
            wvT_t = p1.tile([128, 8, CH], f32r)
            vbb_t = p1.tile([128, CH], f32)
            qkb_t = p1.tile([128, 8], f32)
            nc.sync.dma_start(tokT_t[:], tokT.rearrange("(t p) n -> p t n", p=128))
            nc.sync.dma_start(wqkT_t[:], wqkT.rearrange("(t p) c -> p t c", p=128))
            nc.sync.dma_start(wvT_t[:], wvT.rearrange("(t p) c -> p t c", p=128))
            nc.sync.dma_start(vbb_t[:], vbb[:])
            nc.sync.dma_start(qkb_t[:], qkb.rearrange("(t p) -> p t", p=128))

            for mt in range(8):
                ps = ps_qk.tile([128, N], f32)
                for (qo, qw) in QC:
                    for kt in range(8):
                        nc.tensor.matmul(
                            ps[:, qo:qo + qw],
                            wqkT_t[:, kt, mt * 128:(mt + 1) * 128],
                            tokT_t[:, kt, qo:qo + qw],
                            start=(kt == 0), stop=(kt == 7),
                        )
                nc.scalar.activation(
                    qkT_t[:, mt, :], ps[:], AF.Identity, bias=qkb_t[:, mt:mt + 1]
                )
            for tt in range(NKT):
                tw = 128 if tt < 10 else 90
                psv = ps_v.tile([128, CH], f32)
                for kt in range(8):
                    nc.tensor.matmul(
                        psv[:tw, :],
                        tokT_t[:, kt, tt * 128:tt * 128 + tw],
                        wvT_t[:, kt, :],
                        start=(kt == 0), stop=(kt == 7),
                    )
                nc.vector.tensor_tensor(
                    out=vt_heads[:tw, tt, :, 0:64],
                    in0=psv[:tw, :].rearrange("p (h x) -> p h x", x=64),
                    in1=vbb_t[:tw, :].rearrange("p (h x) -> p h x", x=64),
                    op=ALU.add,
                )

        expp = ctx.enter_context(tc.tile_pool(name="expp", bufs=24))
        small = ctx.enter_context(tc.tile_pool(name="small", bufs=2))
        finp = ctx.enter_context(tc.tile_pool(name="finp", bufs=3))
        ps2 = ctx.enter_context(ExitStack())
        ps_s = ps2.enter_context(tc.tile_pool(name="ps_s", bufs=2, space="PSUM"))
        ps_pv = ps2.enter_context(tc.tile_pool(name="ps_pv", bufs=1, space="PSUM"))
        ps_bc = ps2.enter_context(tc.tile_pool(name="ps_bc", bufs=1, space="PSUM"))

        # ---- phase 2: attention per head pair ----
        for j in range(4):
            exps = {0: [], 1: []}
            for kt in range(NKT):
                kw = 128 if kt < 10 else 90
                for half in (0, 1):
                    r0, r1 = 64 * half, 64 * half + 64
                    ps = ps_s.tile([128, N], f32)
                    for (qo, qw) in QC:
                        nc.tensor.matmul(
                            ps[:kw, qo:qo + qw],
                            qkT_t[r0:r1, 4 + j, kt * 128:kt * 128 + kw],
                            qkT_t[r0:r1, j, qo:qo + qw],
                        )
                    e = expp.tile([128, N], bf16)
                    nc.scalar.activation(e[:kw, :], ps[:kw, :], AF.Exp)
                    exps[half].append(e)
            for half in (0, 1):
                h = 2 * j + half
                for (qo, qw) in QC:
                    pv = ps_pv.tile([65, 512], f32)
                    for kt in range(NKT):
                        kw = 128 if kt < 10 else 90
                        nc.tensor.matmul(
                            pv[:, :qw],
                            v_t[:kw, kt, h * 65:(h + 1) * 65],
                            exps[half][kt][:kw, qo:qo + qw],
                            start=(kt == 0), stop=(kt == NKT - 1),
                        )
                    rec = small.tile([65, 512], f32r, tag="rec")
                    with nc.allow_low_precision(reason="f32r recip rounding"):
                        nc.vector.reciprocal(rec[64:65, :qw], pv[64:65, :qw])
                    bc = ps_bc.tile([64, 512], f32)
                    nc.tensor.matmul(bc[:, :qw], ones_t[64:65, :], rec[64:65, :qw])
                    bcs = small.tile([64, 512], f32, tag="bcs")
                    nc.vector.tensor_copy(bcs[:, :qw], bc[:, :qw])
                    if half == 0:
                        nc.vector.tensor_tensor(
                            out=outT_t[0:64, j, qo:qo + qw],
                            in0=pv[0:64, :qw], in1=bcs[:, :qw], op=ALU.mult,
                        )
                    else:
                        tmpb = small.tile([64, 512], f32r, tag="tmpb")
                        nc.vector.tensor_tensor(
                            out=tmpb[:, :qw],
                            in0=pv[0:64, :qw], in1=bcs[:, :qw], op=ALU.mult,
                        )
                        nc.sync.dma_start(
                            outT_t[64:128, j, qo:qo + qw], tmpb[:, :qw]
                        )

        ps2.close()
        ps_pj = ctx.enter_context(tc.tile_pool(name="ps_pj", bufs=2, space="PSUM"))

        # ---- phase 3: projection partial ----
        for ct in range(8):
            for i, (qo, qw) in enumerate(QC):
                pp = ps_pj.tile([128, 512], f32)
                for j in range(4):
                    nc.tensor.matmul(
                        pp[:, :qw],
                        pwT_t[:, j, ct * 128:(ct + 1) * 128],
                        outT_t[:, j, qo:qo + qw],
                        start=(j == 0), stop=(j == 3),
                    )
                fin = finp.tile([128, 512], f32)
                if ct % 2 == 0:
                    nc.vector.tensor_copy(fin[:, :qw], pp[:, :qw])
                else:
                    nc.scalar.copy(fin[:, :qw], pp[:, :qw])
                nc.sync.dma_start(
                    outT[ct * 128:(ct + 1) * 128, qo:qo + qw], fin[:, :qw]
                )

    _legalize_multiwait(nc, mybir)
    return nc


def _build_nc_cached():
    if "nc" not in _NC_CACHE:
        import concourse.bass as bass  # noqa: F401  (env check)
        # pwT_t is defined inside build_nc; fix forward ref by building there.
        _NC_CACHE["nc"] = build_nc()
    return _NC_CACHE["nc"]


def _prep_group(qkv_w, qkv_b, proj_w, g):
    scale = np.float32(HD ** -0.5)
    qs = slice(CH * g, CH * g + CH)
    ks = slice(1024 + CH * g, 1024 + CH * g + CH)
    vs = slice(2048 + CH * g, 2048 + CH * g + CH)
    wqk = np.concatenate([qkv_w[qs] * scale, qkv_w[ks]], axis=0)
    return {
        "wqkT": np.ascontiguousarray(wqk.T, dtype=np.float32),
        "qkb": np.concatenate([qkv_b[qs] * scale, qkv_b[ks]]).astype(np.float32),
        "wvT": np.ascontiguousarray(qkv_w[vs].T, dtype=np.float32),
        "vbb": np.ascontiguousarray(
            np.broadcast_to(qkv_b[vs], (128, CH)), dtype=np.float32
        ),
        "pwT": np.ascontiguousarray(proj_w[:, CH * g:CH * g + CH].T, dtype=np.float32),
        "onesd": np.ones((128, 64), np.float32),
    }


def kernel(tokens, qkv_w, qkv_b, proj_w, proj_b):
    from concourse.bass_utils import run_bass_kernel_spmd

    tokens = np.asarray(tokens, dtype=np.float32)
    qkv_w = np.asarray(qkv_w, dtype=np.float32)
    qkv_b = np.asarray(qkv_b, dtype=np.float32)
    proj_w = np.asarray(proj_w, dtype=np.float32)
    proj_b = np.asarray(proj_b, dtype=np.float32)

    nc = _build_nc_cached()
    groups = [_prep_group(qkv_w, qkv_b, proj_w, g) for g in range(2)]
    in_maps = []
    for core in range(8):
        b, g = core // 2, core % 2
        m = dict(groups[g])
        m["tokT"] = np.ascontiguousarray(tokens[b].T, dtype=np.float32)
        in_maps.append(m)

    res = run_bass_kernel_spmd(nc, in_maps, core_ids=list(range(8)))
    final = np.empty((B, N, C), np.float32)
    for b in range(B):
        acc = res.results[2 * b]["outT"] + res.results[2 * b + 1]["outT"]
        final[b] = acc.T + proj_b
    return final


# revision 9
# speedup vs baseline: 1.1237x; 1.1237x over previous
"""Multi-head attention block (B=4, N=1370, C=1024, H=16) on 8 NeuronCores.

Sharding: core i -> batch i//2, head-group i%2 (8 heads = 512 channels).
Each core computes qkv^T = W_shard @ tok_b^T, per-head attention in the
transposed (S^T) layout with the softmax denominator folded into the PV
matmul as an extra ones column, then the projection partial
final^T = proj_w[:, shard].T-slice @ out^T.  Host sums the two partials
per batch and adds proj_b.

dtypes: float32r (1 cyc/row, ~1.6e-4 rel) for qkv/proj matmuls; fp16 for
q/k/exp(S)/V (attention operands).
"""

import numpy as np

B, N, C = 4, 1370, 1024
H_PER_CORE = 8
HD = 64          # head dim
CH = 512         # channels per core
NKT = 11         # 128-key tiles (10*128 + 90)
QC = [(0, 512), (512, 512), (1024, 346)]  # query free-dim chunks (bank aligned)

_NC_CACHE = {}


def _legalize_multiwait(nc, mybir):
    """This walrus build accepts only one sync wait per instruction; Tile's
    exit drain stacks one wait per live semaphore.  Hoist extras onto no-ops
    inserted just before the offending instruction."""
    for f in nc.m.functions:
        for bb in f.blocks:
            insts = bb.instructions
            i = 0
            while i < len(insts):
                inst = insts[i]
                si = inst.sync_info
                if si is not None and len(si.on_wait) > 1:
                    waits = list(si.on_wait)
                    for j, w in enumerate(waits[:-1]):
                        nop = mybir.InstNoOp(
                            name=f"{inst.name}-waitsplit-{j}", ins=[], outs=[]
                        )
                        nop.engine = inst.engine
                        nop.sync_info = mybir.SyncInfo(on_wait=[w], on_update=[])
                        insts.insert(i, nop)
                        nc.register_instruction(nop, overwrite=True)
                        i += 1
                    inst.sync_info = mybir.SyncInfo(
                        on_wait=[waits[-1]], on_update=list(si.on_update)
                    )
                i += 1


def build_nc():
    import concourse.bass as bass
    import concourse.mybir as mybir
    import concourse.tile as tile
    from contextlib import ExitStack

    f32 = mybir.dt.float32
    f32r = mybir.dt.float32r
    f16 = mybir.dt.float16
    AF = mybir.ActivationFunctionType
    ALU = mybir.AluOpType

    nc = bass.Bass()

    tokT = nc.dram_tensor("tokT", [C, N], f32r, kind="ExternalInput")
    wqkT = nc.dram_tensor("wqkT", [C, 1024], f32r, kind="ExternalInput")
    qkb = nc.dram_tensor("qkb", [1024], f32, kind="ExternalInput")
    wvT = nc.dram_tensor("wvT", [C, CH], f32r, kind="ExternalInput")
    vbb = nc.dram_tensor("vbb", [128, CH], f32, kind="ExternalInput")
    pwT = nc.dram_tensor("pwT", [CH, 1024], f32r, kind="ExternalInput")
    onesd = nc.dram_tensor("onesd", [128, 64], f32r, kind="ExternalInput")
    outT = nc.dram_tensor("outT", [1024, N], f32, kind="ExternalOutput")

    with tile.TileContext(nc) as tc, ExitStack() as ctx:
        persist = ctx.enter_context(tc.tile_pool(name="persist", bufs=1))

        ones_t = persist.tile([128, 64], f32r)
        pwT_t = persist.tile([128, 4, 1024], f32r)
        qkT_t = persist.tile([128, 8, N], f16)   # q ch 0-511 (mt 0-3), k ch (mt 4-7)
        v_t = persist.tile([128, NKT, 8 * 65], f16)  # per head: 64 V cols + ones col
        outT_t = persist.tile([128, 4, N], f32r)  # attention out^T per pair
        nc.sync.dma_start(pwT_t[:], pwT.rearrange("(t p) c -> p t c", p=128))
        nc.sync.dma_start(ones_t[:], onesd[:])
        vt_heads = v_t[:].rearrange("p t (h x) -> p t h x", x=65)
        nc.vector.memset(vt_heads[:, :, :, 64:65], 1.0)

        # ---- phase 1: qkv projection (scoped pools, freed afterwards) ----
        with tc.tile_pool(name="p1", bufs=1) as p1, \
             tc.tile_pool(name="ps_qk", bufs=2, space="PSUM") as ps_qk, \
             tc.tile_pool(name="ps_v", bufs=2, space="PSUM") as ps_v:
            tokT_t = p1.tile([128, 8, N], f32r)
            wqkT_t = p1.tile([128, 8, 1024], f32r)
            wvT_t = p1.tile([128, 8, CH], f32r)
            vbb_t = p1.tile([128, CH], f32)
            qkb_t = p1.tile([128, 8], f32)
            tokT_r = tokT.rearrange("(t p) n -> p t n", p=128)
            wqkT_r = wqkT.rearrange("(t p) c -> p t c", p=128)
            wvT_r = wvT.rearrange("(t p) c -> p t c", p=128)
            # chunk input DMAs per contraction tile so matmuls start early
            for kt in range(8):
                nc.sync.dma_start(tokT_t[:, kt, :], tokT_r[:, kt, :])
                nc.sync.dma_start(wqkT_t[:, kt, :], wqkT_r[:, kt, :])
                nc.sync.dma_start(wvT_t[:, kt, :], wvT_r[:, kt, :])
            nc.sync.dma_start(vbb_t[:], vbb[:])
            nc.sync.dma_start(qkb_t[:], qkb.rearrange("(t p) -> p t", p=128))

            for mt in range(8):
                ps = ps_qk.tile([128, N], f32)
                for (qo, qw) in QC:
                    for kt in range(8):
                        nc.tensor.matmul(
                            ps[:, qo:qo + qw],
                            wqkT_t[:, kt, mt * 128:(mt + 1) * 128],
                            tokT_t[:, kt, qo:qo + qw],
                            start=(kt == 0), stop=(kt == 7),
                        )
                nc.vector.tensor_scalar_add(
                    qkT_t[:, mt, :], ps[:], qkb_t[:, mt:mt + 1]
                )
            for tt in range(NKT):
                tw = 128 if tt < 10 else 90
                psv = ps_v.tile([128, CH], f32)
                for kt in range(8):
                    nc.tensor.matmul(
                        psv[:tw, :],
                        tokT_t[:, kt, tt * 128:tt * 128 + tw],
                        wvT_t[:, kt, :],
                        start=(kt == 0), stop=(kt == 7),
                    )
                nc.vector.tensor_tensor(
                    out=vt_heads[:tw, tt, :, 0:64],
                    in0=psv[:tw, :].rearrange("p (h x) -> p h x", x=64),
                    in1=vbb_t[:tw, :].rearrange("p (h x) -> p h x", x=64),
                    op=ALU.add,
                )

        expp = ctx.enter_context(tc.tile_pool(name="expp", bufs=26))
        small = ctx.enter_context(tc.tile_pool(name="small", bufs=2))
        finp = ctx.enter_context(tc.tile_pool(name="finp", bufs=6))
        ps2 = ctx.enter_context(ExitStack())
        ps_s = ps2.enter_context(tc.tile_pool(name="ps_s", bufs=2, space="PSUM"))
        ps_pv = ps2.enter_context(tc.tile_pool(name="ps_pv", bufs=1, space="PSUM"))
        ps_bc = ps2.enter_context(tc.tile_pool(name="ps_bc", bufs=1, space="PSUM"))

        # ---- phase 2: attention, software-pipelined across head pairs ----
        # emit order: S(0); S(1); PV(0); S(2); PV(1); S(3); PV(2); PV(3)
        def emit_s_exp(j):
            exps = {0: [], 1: []}
            for kt in range(NKT):
                kw = 128 if kt < 10 else 90
                for half in (0, 1):
                    r0, r1 = 64 * half, 64 * half + 64
                    ps = ps_s.tile([128, N], f32)
                    for (qo, qw) in QC:
                        nc.tensor.matmul(
                            ps[:kw, qo:qo + qw],
                            qkT_t[r0:r1, 4 + j, kt * 128:kt * 128 + kw],
                            qkT_t[r0:r1, j, qo:qo + qw],
                        )
                    e = expp.tile([128, N], f16)
                    nc.scalar.activation(e[:kw, :], ps[:kw, :], AF.Exp)
                    exps[half].append(e)
            return exps

        def emit_pv(j, exps):
            for half in (0, 1):
                h = 2 * j + half
                for (qo, qw) in QC:
                    pv = ps_pv.tile([65, 512], f32)
                    for kt in range(NKT):
                        kw = 128 if kt < 10 else 90
                        nc.tensor.matmul(
                            pv[:, :qw],
                            v_t[:kw, kt, h * 65:(h + 1) * 65],
                            exps[half][kt][:kw, qo:qo + qw],
                            start=(kt == 0), stop=(kt == NKT - 1),
                        )
                    rec = small.tile([65, 512], f32r, tag="rec")
                    with nc.allow_low_precision(reason="f32r recip rounding"):
                        nc.vector.reciprocal(rec[64:65, :qw], pv[64:65, :qw])
                    bc = ps_bc.tile([64, 512], f32)
                    nc.tensor.matmul(bc[:, :qw], ones_t[64:65, :], rec[64:65, :qw])
                    bcs = small.tile([64, 512], f32, tag="bcs")
                    nc.vector.tensor_copy(bcs[:, :qw], bc[:, :qw])
                    if half == 0:
                        nc.vector.tensor_tensor(
                            out=outT_t[0:64, j, qo:qo + qw],
                            in0=pv[0:64, :qw], in1=bcs[:, :qw], op=ALU.mult,
                        )
                    else:
                        tmpb = small.tile([64, 512], f32r, tag="tmpb")
                        nc.vector.tensor_tensor(
                            out=tmpb[:, :qw],
                            in0=pv[0:64, :qw], in1=bcs[:, :qw], op=ALU.mult,
                        )
                        nc.sync.dma_start(
                            outT_t[64:128, j, qo:qo + qw], tmpb[:, :qw]
                        )

        prev = emit_s_exp(0)
        for j in range(1, 4):
            cur = emit_s_exp(j)
            emit_pv(j - 1, prev)
            prev = cur
        emit_pv(3, prev)

        ps2.close()
        ps_pj = ctx.enter_context(tc.tile_pool(name="ps_pj", bufs=4, space="PSUM"))

        # ---- phase 3: projection partial ----
        for ct in range(8):
            for i, (qo, qw) in enumerate(QC):
                pp = ps_pj.tile([128, 512], f32)
                for j in range(4):
                    nc.tensor.matmul(
                        pp[:, :qw],
                        pwT_t[:, j, ct * 128:(ct + 1) * 128],
                        outT_t[:, j, qo:qo + qw],
                        start=(j == 0), stop=(j == 3),
                    )
                fin = finp.tile([128, 512], f32)
                if (ct * 3 + i) % 2 == 0:
                    nc.vector.tensor_copy(fin[:, :qw], pp[:, :qw])
                else:
                    nc.scalar.copy(fin[:, :qw], pp[:, :qw])
                nc.sync.dma_start(
                    outT[ct * 128:(ct + 1) * 128, qo:qo + qw], fin[:, :qw]
                )

    _legalize_multiwait(nc, mybir)
    return nc


def _build_nc_cached():
    if "nc" not in _NC_CACHE:
        _NC_CACHE["nc"] = build_nc()
    return _NC_CACHE["nc"]


def _prep_group(qkv_w, qkv_b, proj_w, g):
    scale = np.float32(HD ** -0.5)
    qs = slice(CH * g, CH * g + CH)
    ks = slice(1024 + CH * g, 1024 + CH * g + CH)
    vs = slice(2048 + CH * g, 2048 + CH * g + CH)
    wqk = np.concatenate([qkv_w[qs] * scale, qkv_w[ks]], axis=0)
    return {
        "wqkT": np.ascontiguousarray(wqk.T, dtype=np.float32),
        "qkb": np.concatenate([qkv_b[qs] * scale, qkv_b[ks]]).astype(np.float32),
        "wvT": np.ascontiguousarray(qkv_w[vs].T, dtype=np.float32),
        "vbb": np.ascontiguousarray(
            np.broadcast_to(qkv_b[vs], (128, CH)), dtype=np.float32
        ),
        "pwT": np.ascontiguousarray(proj_w[:, CH * g:CH * g + CH].T, dtype=np.float32),
        "onesd": np.ones((128, 64), np.float32),
    }


def kernel(tokens, qkv_w, qkv_b, proj_w, proj_b):
    from concourse.bass_utils import run_bass_kernel_spmd

    tokens = np.asarray(tokens, dtype=np.float32)
    qkv_w = np.asarray(qkv_w, dtype=np.float32)
    qkv_b = np.asarray(qkv_b, dtype=np.float32)
    proj_w = np.asarray(proj_w, dtype=np.float32)
    proj_b = np.asarray(proj_b, dtype=np.float32)

    nc = _build_nc_cached()
    groups = [_prep_group(qkv_w, qkv_b, proj_w, g) for g in range(2)]
    in_maps = []
    for core in range(8):
        b, g = core // 2, core % 2
        m = dict(groups[g])
        m["tokT"] = np.ascontiguousarray(tokens[b].T, dtype=np.float32)
        in_maps.append(m)

    res = run_bass_kernel_spmd(nc, in_maps, core_ids=list(range(8)))
    final = np.empty((B, N, C), np.float32)
    for b in range(B):
        acc = res.results[2 * b]["outT"] + res.results[2 * b + 1]["outT"]
        final[b] = acc.T + proj_b
    return final


# revision 13
# speedup vs baseline: 1.2683x; 1.1287x over previous
"""Multi-head attention block (B=4, N=1370, C=1024, H=16) on 8 NeuronCores.

Sharding: core i -> batch i//2, head-group i%2 (8 heads = 512 channels).
Each core computes qkv^T = W_shard @ tok_b^T, per-head attention in the
transposed (S^T) layout with the softmax denominator folded into the PV
matmul as an extra ones column, then the projection partial
final^T = proj_w[:, shard].T-slice @ out^T.  Host sums the two partials
per batch and adds proj_b.

dtypes: fp16 operands for all big matmuls (1 cyc/row); fp32 PSUM
accumulation everywhere; float32r for the softmax reciprocal/broadcast.
"""

import numpy as np

B, N, C = 4, 1370, 1024
H_PER_CORE = 8
HD = 64          # head dim
CH = 512         # channels per core
NKT = 11         # 128-key tiles (10*128 + 90)
QC = [(0, 512), (512, 512), (1024, 346)]  # query free-dim chunks (bank aligned)

_NC_CACHE = {}


def _legalize_multiwait(nc, mybir):
    """This walrus build accepts only one sync wait per instruction; Tile's
    exit drain stacks one wait per live semaphore.  Hoist extras onto no-ops
    inserted just before the offending instruction."""
    for f in nc.m.functions:
        for bb in f.blocks:
            insts = bb.instructions
            i = 0
            while i < len(insts):
                inst = insts[i]
                si = inst.sync_info
                if si is not None and len(si.on_wait) > 1:
                    waits = list(si.on_wait)
                    for j, w in enumerate(waits[:-1]):
                        nop = mybir.InstNoOp(
                            name=f"{inst.name}-waitsplit-{j}", ins=[], outs=[]
                        )
                        nop.engine = inst.engine
                        nop.sync_info = mybir.SyncInfo(on_wait=[w], on_update=[])
                        insts.insert(i, nop)
                        nc.register_instruction(nop, overwrite=True)
                        i += 1
                    inst.sync_info = mybir.SyncInfo(
                        on_wait=[waits[-1]], on_update=list(si.on_update)
                    )
                i += 1


def _interleave(a, b):
    """Merge emission thunk lists roughly proportionally (a paced over b)."""
    out = []
    ia = ib = 0
    while ia < len(a) or ib < len(b):
        ra = ia / max(len(a), 1)
        rb = ib / max(len(b), 1)
        if ib >= len(b) or (ia < len(a) and ra <= rb):
            out.append(a[ia]); ia += 1
        else:
            out.append(b[ib]); ib += 1
    return out


def build_nc():
    import concourse.bass as bass
    import concourse.mybir as mybir
    import concourse.tile as tile
    from contextlib import ExitStack

    f32 = mybir.dt.float32
    f32r = mybir.dt.float32r
    f16 = mybir.dt.float16
    AF = mybir.ActivationFunctionType
    ALU = mybir.AluOpType

    nc = bass.Bass()

    tokT = nc.dram_tensor("tokT", [C, N], f16, kind="ExternalInput")
    wqkT = nc.dram_tensor("wqkT", [C, 1024], f16, kind="ExternalInput")
    qkb = nc.dram_tensor("qkb", [1024], f32, kind="ExternalInput")
    wvT = nc.dram_tensor("wvT", [C, CH], f16, kind="ExternalInput")
    vbb = nc.dram_tensor("vbb", [128, CH], f32, kind="ExternalInput")
    pwT = nc.dram_tensor("pwT", [CH, 1024], f16, kind="ExternalInput")
    onesd = nc.dram_tensor("onesd", [128, 64], f32r, kind="ExternalInput")
    outT = nc.dram_tensor("outT", [1024, N], f32, kind="ExternalOutput")

    with tile.TileContext(nc) as tc, ExitStack() as ctx:
        persist = ctx.enter_context(tc.tile_pool(name="persist", bufs=1))
        p1 = ctx.enter_context(tc.tile_pool(name="p1", bufs=1))
        expp = ctx.enter_context(tc.tile_pool(name="expp", bufs=28))
        small = ctx.enter_context(tc.tile_pool(name="small", bufs=2))
        finp = ctx.enter_context(tc.tile_pool(name="finp", bufs=6))
        ps_att = ExitStack()
        ps_s = ps_att.enter_context(tc.tile_pool(name="ps_s", bufs=2, space="PSUM"))
        ps_qkv_stack = ExitStack()
        ps_qkv = ps_qkv_stack.enter_context(
            tc.tile_pool(name="ps_qkv", bufs=2, space="PSUM"))

        ones_t = persist.tile([128, 64], f32r)
        pwT_t = persist.tile([128, 4, 1024], f16)
        qkT_t = persist.tile([128, 8, N], f16)   # q ch 0-511 (mt 0-3), k ch (mt 4-7)
        v_t = persist.tile([128, NKT, 8 * 65], f16)  # per head: 64 V cols + ones col
        outT_t = persist.tile([128, 4, N], f16)  # attention out^T per pair

        tokT_t = p1.tile([128, 8, N], f16)
        wqkT_t = p1.tile([128, 8, 1024], f16)
        wvT_t = p1.tile([128, 8, CH], f16)
        vbb_t = p1.tile([128, CH], f32)
        qkb_t = p1.tile([128, 8], f32)
        tokT_r = tokT.rearrange("(t p) n -> p t n", p=128)
        wqkT_r = wqkT.rearrange("(t p) c -> p t c", p=128)
        wvT_r = wvT.rearrange("(t p) c -> p t c", p=128)
        for kt in range(8):
            nc.sync.dma_start(tokT_t[:, kt, :], tokT_r[:, kt, :])
            nc.sync.dma_start(wqkT_t[:, kt, :], wqkT_r[:, kt, :])
        for kt in range(8):
            nc.sync.dma_start(wvT_t[:, kt, :], wvT_r[:, kt, :])
        nc.sync.dma_start(vbb_t[:], vbb[:])
        nc.sync.dma_start(qkb_t[:], qkb.rearrange("(t p) -> p t", p=128))
        nc.sync.dma_start(ones_t[:], onesd[:])
        nc.sync.dma_start(pwT_t[:], pwT.rearrange("(t p) c -> p t c", p=128))
        vt_heads = v_t[:].rearrange("p t (h x) -> p t h x", x=65)
        nc.vector.memset(vt_heads[:, :, :, 64:65], 1.0)

        # ---- emission thunks ----
        def qk_unit(mt, qo, qw):
            def emit():
                ps = ps_qkv.tile([128, 512], f32, tag="qkv")
                for kt in range(8):
                    nc.tensor.matmul(
                        ps[:, :qw],
                        wqkT_t[:, kt, mt * 128:(mt + 1) * 128],
                        tokT_t[:, kt, qo:qo + qw],
                        start=(kt == 0), stop=(kt == 7),
                    )
                nc.vector.tensor_scalar_add(
                    qkT_t[:, mt, qo:qo + qw], ps[:, :qw], qkb_t[:, mt:mt + 1]
                )
            return emit

        def v_unit(tt):
            def emit():
                tw = 128 if tt < 10 else 90
                psv = ps_qkv.tile([128, 512], f32, tag="qkv")
                for kt in range(8):
                    nc.tensor.matmul(
                        psv[:tw, :],
                        tokT_t[:, kt, tt * 128:tt * 128 + tw],
                        wvT_t[:, kt, :],
                        start=(kt == 0), stop=(kt == 7),
                    )
                nc.vector.tensor_tensor(
                    out=vt_heads[:tw, tt, :, 0:64],
                    in0=psv[:tw, :].rearrange("p (h x) -> p h x", x=64),
                    in1=vbb_t[:tw, :].rearrange("p (h x) -> p h x", x=64),
                    op=ALU.add,
                )
            return emit

        exps = {}  # (j, half) -> [expS tiles per kt]

        def s_unit(j, kt, half):
            def emit():
                kw = 128 if kt < 10 else 90
                r0, r1 = 64 * half, 64 * half + 64
                ps = ps_s.tile([128, N], f32)
                for (qo, qw) in QC:
                    nc.tensor.matmul(
                        ps[:kw, qo:qo + qw],
                        qkT_t[r0:r1, 4 + j, kt * 128:kt * 128 + kw],
                        qkT_t[r0:r1, j, qo:qo + qw],
                    )
                e = expp.tile([128, N], f16)
                nc.scalar.activation(e[:kw, :], ps[:kw, :], AF.Exp)
                exps.setdefault((j, half), []).append(e)
            return emit

        def pv_unit(j, half, qo, qw):
            def emit():
                h = 2 * j + half
                elist = exps[(j, half)]
                pv = ps_pv.tile([65, 512], f32)
                for kt in range(NKT):
                    kw = 128 if kt < 10 else 90
                    nc.tensor.matmul(
                        pv[:, :qw],
                        v_t[:kw, kt, h * 65:(h + 1) * 65],
                        elist[kt][:kw, qo:qo + qw],
                        start=(kt == 0), stop=(kt == NKT - 1),
                    )
                rec = small.tile([65, 512], f32r, tag="rec")
                with nc.allow_low_precision(reason="f32r recip rounding"):
                    nc.vector.reciprocal(rec[64:65, :qw], pv[64:65, :qw])
                bc = ps_bc.tile([64, 512], f32)
                nc.tensor.matmul(bc[:, :qw], ones_t[64:65, :], rec[64:65, :qw])
                bcs = small.tile([64, 512], f32, tag="bcs")
                nc.vector.tensor_copy(bcs[:, :qw], bc[:, :qw])
                if half == 0:
                    nc.vector.tensor_tensor(
                        out=outT_t[0:64, j, qo:qo + qw],
                        in0=pv[0:64, :qw], in1=bcs[:, :qw], op=ALU.mult,
                    )
                else:
                    tmpb = small.tile([64, 512], f16, tag="tmpb")
                    nc.vector.tensor_tensor(
                        out=tmpb[:, :qw],
                        in0=pv[0:64, :qw], in1=bcs[:, :qw], op=ALU.mult,
                    )
                    nc.sync.dma_start(outT_t[64:128, j, qo:qo + qw], tmpb[:, :qw])
            return emit

        def s_units(j):
            return [s_unit(j, kt, half) for kt in range(NKT) for half in (0, 1)]

        def pv_units(j):
            return [pv_unit(j, half, qo, qw) for half in (0, 1) for (qo, qw) in QC]

        # ---- emission schedule ----
        # qk tiles 0 and 4 first (pair 0 operands), then S(0) interleaved with
        # the rest of phase 1; then S(j+1) interleaved with PV(j).
        for mt in (0, 4):
            for (qo, qw) in QC:
                qk_unit(mt, qo, qw)()
        rest = [qk_unit(mt, qo, qw) for mt in (1, 5, 2, 6, 3, 7) for (qo, qw) in QC]
        rest += [v_unit(tt) for tt in range(NKT)]
        for th in _interleave(s_units(0), rest):
            th()
        ps_qkv_stack.close()
        ps_pv = ps_att.enter_context(tc.tile_pool(name="ps_pv", bufs=1, space="PSUM"))
        ps_bc = ps_att.enter_context(tc.tile_pool(name="ps_bc", bufs=1, space="PSUM"))
        for j in range(1, 4):
            # spread PV(j-1) over the first 2/3 of S(j)
            su = s_units(j)
            cut = (2 * len(su)) // 3
            for th in _interleave(su[:cut], pv_units(j - 1)):
                th()
            for th in su[cut:]:
                th()
        for th in pv_units(3):
            th()

        ps_att.close()
        ps_pj = ctx.enter_context(tc.tile_pool(name="ps_pj", bufs=4, space="PSUM"))

        # ---- projection partial ----
        for ct in range(8):
            for i, (qo, qw) in enumerate(QC):
                pp = ps_pj.tile([128, 512], f32)
                for j in range(4):
                    nc.tensor.matmul(
                        pp[:, :qw],
                        pwT_t[:, j, ct * 128:(ct + 1) * 128],
                        outT_t[:, j, qo:qo + qw],
                        start=(j == 0), stop=(j == 3),
                    )
                fin = finp.tile([128, 512], f32)
                if (ct * 3 + i) % 2 == 0:
                    nc.vector.tensor_copy(fin[:, :qw], pp[:, :qw])
                else:
                    nc.scalar.copy(fin[:, :qw], pp[:, :qw])
                nc.sync.dma_start(
                    outT[ct * 128:(ct + 1) * 128, qo:qo + qw], fin[:, :qw]
                )

    _legalize_multiwait(nc, mybir)
    return nc


def _build_nc_cached():
    if "nc" not in _NC_CACHE:
        _NC_CACHE["nc"] = build_nc()
    return _NC_CACHE["nc"]


def _prep_group(qkv_w, qkv_b, proj_w, g):
    scale = np.float32(HD ** -0.5)
    qs = slice(CH * g, CH * g + CH)
    ks = slice(1024 + CH * g, 1024 + CH * g + CH)
    vs = slice(2048 + CH * g, 2048 + CH * g + CH)
    wqk = np.concatenate([qkv_w[qs] * scale, qkv_w[ks]], axis=0)
    return {
        "wqkT": np.ascontiguousarray(wqk.T, dtype=np.float16),
        "qkb": np.concatenate([qkv_b[qs] * scale, qkv_b[ks]]).astype(np.float32),
        "wvT": np.ascontiguousarray(qkv_w[vs].T, dtype=np.float16),
        "vbb": np.ascontiguousarray(
            np.broadcast_to(qkv_b[vs], (128, CH)), dtype=np.float32
        ),
        "pwT": np.ascontiguousarray(proj_w[:, CH * g:CH * g + CH].T, dtype=np.float16),
        "onesd": np.ones((128, 64), np.float32),
    }


def kernel(tokens, qkv_w, qkv_b, proj_w, proj_b):
    from concourse.bass_utils import run_bass_kernel_spmd

    tokens = np.asarray(tokens, dtype=np.float32)
    qkv_w = np.asarray(qkv_w, dtype=np.float32)
    qkv_b = np.asarray(qkv_b, dtype=np.float32)
    proj_w = np.asarray(proj_w, dtype=np.float32)
    proj_b = np.asarray(proj_b, dtype=np.float32)

    nc = _build_nc_cached()
    groups = [_prep_group(qkv_w, qkv_b, proj_w, g) for g in range(2)]
    in_maps = []
    for core in range(8):
        b, g = core // 2, core % 2
        m = dict(groups[g])
        m["tokT"] = np.ascontiguousarray(tokens[b].T, dtype=np.float16)
        in_maps.append(m)

    res = run_bass_kernel_spmd(nc, in_maps, core_ids=list(range(8)))
    final = np.empty((B, N, C), np.float32)
    for b in range(B):
        acc = res.results[2 * b]["outT"] + res.results[2 * b + 1]["outT"]
        final[b] = acc.T + proj_b
    return final


# revision 14
# speedup vs baseline: 1.3123x; 1.0347x over previous
"""Multi-head attention block (B=4, N=1370, C=1024, H=16) on 8 NeuronCores.

Sharding: core i -> batch i//2, head-group i%2 (8 heads = 512 channels).
Each core computes qkv^T = W_shard @ tok_b^T, per-head attention in the
transposed (S^T) layout with the softmax denominator folded into the PV
matmul as an extra ones column, then the projection partial
final^T = proj_w[:, shard].T-slice @ out^T.  Host sums the two partials
per batch and adds proj_b.

dtypes: fp16 operands for all big matmuls (1 cyc/row); fp32 PSUM
accumulation everywhere; float32r for the softmax reciprocal/broadcast.
"""

import numpy as np

B, N, C = 4, 1370, 1024
H_PER_CORE = 8
HD = 64          # head dim
CH = 512         # channels per core
NKT = 11         # 128-key tiles (10*128 + 90)
QC = [(0, 512), (512, 512), (1024, 346)]  # query free-dim chunks (bank aligned)

_NC_CACHE = {}


def _legalize_multiwait(nc, mybir):
    """This walrus build accepts only one sync wait per instruction; Tile's
    exit drain stacks one wait per live semaphore.  Hoist extras onto no-ops
    inserted just before the offending instruction."""
    for f in nc.m.functions:
        for bb in f.blocks:
            insts = bb.instructions
            i = 0
            while i < len(insts):
                inst = insts[i]
                si = inst.sync_info
                if si is not None and len(si.on_wait) > 1:
                    waits = list(si.on_wait)
                    for j, w in enumerate(waits[:-1]):
                        nop = mybir.InstNoOp(
                            name=f"{inst.name}-waitsplit-{j}", ins=[], outs=[]
                        )
                        nop.engine = inst.engine
                        nop.sync_info = mybir.SyncInfo(on_wait=[w], on_update=[])
                        insts.insert(i, nop)
                        nc.register_instruction(nop, overwrite=True)
                        i += 1
                    inst.sync_info = mybir.SyncInfo(
                        on_wait=[waits[-1]], on_update=list(si.on_update)
                    )
                i += 1


def _interleave(a, b):
    """Merge emission thunk lists roughly proportionally (a paced over b)."""
    out = []
    ia = ib = 0
    while ia < len(a) or ib < len(b):
        ra = ia / max(len(a), 1)
        rb = ib / max(len(b), 1)
        if ib >= len(b) or (ia < len(a) and ra <= rb):
            out.append(a[ia]); ia += 1
        else:
            out.append(b[ib]); ib += 1
    return out


def build_nc():
    import concourse.bass as bass
    import concourse.mybir as mybir
    import concourse.tile as tile
    from contextlib import ExitStack

    f32 = mybir.dt.float32
    f32r = mybir.dt.float32r
    f16 = mybir.dt.float16
    AF = mybir.ActivationFunctionType
    ALU = mybir.AluOpType

    nc = bass.Bass()

    tokT = nc.dram_tensor("tokT", [C, N], f16, kind="ExternalInput")
    wqkT = nc.dram_tensor("wqkT", [C, 1024], f16, kind="ExternalInput")
    qkb = nc.dram_tensor("qkb", [1024], f32, kind="ExternalInput")
    wvT = nc.dram_tensor("wvT", [C, CH], f16, kind="ExternalInput")
    vbb = nc.dram_tensor("vbb", [128, CH], f32, kind="ExternalInput")
    pwT = nc.dram_tensor("pwT", [CH, 1024], f16, kind="ExternalInput")
    onesd = nc.dram_tensor("onesd", [128, 64], f32r, kind="ExternalInput")
    outT = nc.dram_tensor("outT", [1024, N], f32, kind="ExternalOutput")

    with tile.TileContext(nc) as tc, ExitStack() as ctx:
        persist = ctx.enter_context(tc.tile_pool(name="persist", bufs=1))
        p1 = ctx.enter_context(tc.tile_pool(name="p1", bufs=1))
        expp = ctx.enter_context(tc.tile_pool(name="expp", bufs=28))
        small = ctx.enter_context(tc.tile_pool(name="small", bufs=2))
        finp = ctx.enter_context(tc.tile_pool(name="finp", bufs=6))
        ps_att = ExitStack()
        ps_s = ps_att.enter_context(tc.tile_pool(name="ps_s", bufs=2, space="PSUM"))
        ps_qkv_stack = ExitStack()
        ps_qkv = ps_qkv_stack.enter_context(
            tc.tile_pool(name="ps_qkv", bufs=2, space="PSUM"))

        ones_t = persist.tile([128, 64], f32r)
        pwT_t = persist.tile([128, 4, 1024], f16)
        qkT_t = persist.tile([128, 8, N], f16)   # q ch 0-511 (mt 0-3), k ch (mt 4-7)
        v_t = persist.tile([128, NKT, 8 * 65], f16)  # per head: 64 V cols + ones col
        outT_t = persist.tile([128, 4, N], f16)  # attention out^T per pair

        tokT_t = p1.tile([128, 8, N], f16)
        wqkT_t = p1.tile([128, 8, 1024], f16)
        wvT_t = p1.tile([128, 8, CH], f16)
        vbb_t = p1.tile([128, CH], f32)
        qkb_t = p1.tile([128, 8], f32)
        tokT_r = tokT.rearrange("(t p) n -> p t n", p=128)
        wqkT_r = wqkT.rearrange("(t p) c -> p t c", p=128)
        wvT_r = wvT.rearrange("(t p) c -> p t c", p=128)
        for kt in range(8):
            nc.sync.dma_start(tokT_t[:, kt, :], tokT_r[:, kt, :])
            nc.sync.dma_start(wqkT_t[:, kt, :], wqkT_r[:, kt, :])
        for kt in range(8):
            nc.sync.dma_start(wvT_t[:, kt, :], wvT_r[:, kt, :])
        nc.sync.dma_start(vbb_t[:], vbb[:])
        nc.sync.dma_start(qkb_t[:], qkb.rearrange("(t p) -> p t", p=128))
        nc.sync.dma_start(ones_t[:], onesd[:])
        nc.sync.dma_start(pwT_t[:], pwT.rearrange("(t p) c -> p t c", p=128))
        vt_heads = v_t[:].rearrange("p t (h x) -> p t h x", x=65)
        nc.vector.memset(vt_heads[:, :, :, 64:65], 1.0)

        # ---- emission thunks ----
        def qk_unit(mt, qo, qw):
            def emit():
                ps = ps_qkv.tile([128, 512], f32, tag="qkv")
                for kt in range(8):
                    nc.tensor.matmul(
                        ps[:, :qw],
                        wqkT_t[:, kt, mt * 128:(mt + 1) * 128],
                        tokT_t[:, kt, qo:qo + qw],
                        start=(kt == 0), stop=(kt == 7),
                    )
                nc.vector.tensor_scalar_add(
                    qkT_t[:, mt, qo:qo + qw], ps[:, :qw], qkb_t[:, mt:mt + 1]
                )
            return emit

        def v_unit(tt):
            def emit():
                tw = 128 if tt < 10 else 90
                psv = ps_qkv.tile([128, 512], f32, tag="qkv")
                for kt in range(8):
                    nc.tensor.matmul(
                        psv[:tw, :],
                        tokT_t[:, kt, tt * 128:tt * 128 + tw],
                        wvT_t[:, kt, :],
                        start=(kt == 0), stop=(kt == 7),
                    )
                nc.vector.tensor_tensor(
                    out=vt_heads[:tw, tt, :, 0:64],
                    in0=psv[:tw, :].rearrange("p (h x) -> p h x", x=64),
                    in1=vbb_t[:tw, :].rearrange("p (h x) -> p h x", x=64),
                    op=ALU.add,
                )
            return emit

        exps = {}  # (j, half) -> [expS tiles per kt]

        def s_unit(j, kt, half):
            def emit():
                kw = 128 if kt < 10 else 90
                r0, r1 = 64 * half, 64 * half + 64
                ps = ps_s.tile([128, N], f32)
                for (qo, qw) in QC:
                    nc.tensor.matmul(
                        ps[:kw, qo:qo + qw],
                        qkT_t[r0:r1, 4 + j, kt * 128:kt * 128 + kw],
                        qkT_t[r0:r1, j, qo:qo + qw],
                    )
                e = expp.tile([128, N], f16)
                nc.scalar.activation(e[:kw, :], ps[:kw, :], AF.Exp)
                exps.setdefault((j, half), []).append(e)
            return emit

        def pv_unit(j, half, qo, qw):
            def emit():
                h = 2 * j + half
                elist = exps[(j, half)]
                pv = ps_pv.tile([65, 512], f32)
                for kt in range(NKT):
                    kw = 128 if kt < 10 else 90
                    nc.tensor.matmul(
                        pv[:, :qw],
                        v_t[:kw, kt, h * 65:(h + 1) * 65],
                        elist[kt][:kw, qo:qo + qw],
                        start=(kt == 0), stop=(kt == NKT - 1),
                    )
                rec = small.tile([65, 512], f32r, tag="rec")
                with nc.allow_low_precision(reason="f32r recip rounding"):
                    nc.vector.reciprocal(rec[64:65, :qw], pv[64:65, :qw])
                raw = small.tile([64, 512], f32, tag="raw")
                nc.vector.tensor_copy(raw[:, :qw], pv[0:64, :qw])
                bc = ps_bc.tile([64, 512], f32)
                nc.tensor.matmul(bc[:, :qw], ones_t[64:65, :], rec[64:65, :qw])
                if half == 0:
                    nc.vector.tensor_tensor(
                        out=outT_t[0:64, j, qo:qo + qw],
                        in0=bc[:, :qw], in1=raw[:, :qw], op=ALU.mult,
                    )
                else:
                    tmpb = small.tile([64, 512], f16, tag="tmpb")
                    nc.vector.tensor_tensor(
                        out=tmpb[:, :qw],
                        in0=bc[:, :qw], in1=raw[:, :qw], op=ALU.mult,
                    )
                    nc.sync.dma_start(outT_t[64:128, j, qo:qo + qw], tmpb[:, :qw])
            return emit

        def s_units(j):
            return [s_unit(j, kt, half) for kt in range(NKT) for half in (0, 1)]

        def pv_units(j):
            return [pv_unit(j, half, qo, qw) for (qo, qw) in QC for half in (0, 1)]

        # ---- emission schedule ----
        # qk tiles 0 and 4 first (pair 0 operands), then S(0) interleaved with
        # the rest of phase 1; then S(j+1) interleaved with PV(j).
        for mt in (0, 4):
            for (qo, qw) in QC:
                qk_unit(mt, qo, qw)()
        rest = [qk_unit(mt, qo, qw) for mt in (1, 5, 2, 6, 3, 7) for (qo, qw) in QC]
        rest += [v_unit(tt) for tt in range(NKT)]
        for th in _interleave(s_units(0), rest):
            th()
        ps_qkv_stack.close()
        ps_pv = ps_att.enter_context(tc.tile_pool(name="ps_pv", bufs=1, space="PSUM"))
        ps_bc = ps_att.enter_context(tc.tile_pool(name="ps_bc", bufs=1, space="PSUM"))
        for j in range(1, 4):
            # spread PV(j-1) over the first 2/3 of S(j)
            su = s_units(j)
            cut = (2 * len(su)) // 3
            for th in _interleave(su[:cut], pv_units(j - 1)):
                th()
            for th in su[cut:]:
                th()
        for th in pv_units(3):
            th()

        ps_att.close()
        ps_pj = ctx.enter_context(tc.tile_pool(name="ps_pj", bufs=4, space="PSUM"))

        # ---- projection partial ----
        for i, (qo, qw) in enumerate(QC):
            for ct in range(8):
                pp = ps_pj.tile([128, 512], f32)
                for j in range(4):
                    nc.tensor.matmul(
                        pp[:, :qw],
                        pwT_t[:, j, ct * 128:(ct + 1) * 128],
                        outT_t[:, j, qo:qo + qw],
                        start=(j == 0), stop=(j == 3),
                    )
                fin = finp.tile([128, 512], f32)
                if (ct * 3 + i) % 2 == 0:
                    nc.vector.tensor_copy(fin[:, :qw], pp[:, :qw])
                else:
                    nc.scalar.copy(fin[:, :qw], pp[:, :qw])
                nc.sync.dma_start(
                    outT[ct * 128:(ct + 1) * 128, qo:qo + qw], fin[:, :qw]
                )

    _legalize_multiwait(nc, mybir)
    return nc


def _build_nc_cached():
    if "nc" not in _NC_CACHE:
        _NC_CACHE["nc"] = build_nc()
    return _NC_CACHE["nc"]


def _prep_group(qkv_w, qkv_b, proj_w, g):
    scale = np.float32(HD ** -0.5)
    qs = slice(CH * g, CH * g + CH)
    ks = slice(1024 + CH * g, 1024 + CH * g + CH)
    vs = slice(2048 + CH * g, 2048 + CH * g + CH)
    wqk = np.concatenate([qkv_w[qs] * scale, qkv_w[ks]], axis=0)
    return {
        "wqkT": np.ascontiguousarray(wqk.T, dtype=np.float16),
        "qkb": np.concatenate([qkv_b[qs] * scale, qkv_b[ks]]).astype(np.float32),
        "wvT": np.ascontiguousarray(qkv_w[vs].T, dtype=np.float16),
        "vbb": np.ascontiguousarray(
            np.broadcast_to(qkv_b[vs], (128, CH)), dtype=np.float32
        ),
        "pwT": np.ascontiguousarray(proj_w[:, CH * g:CH * g + CH].T, dtype=np.float16),
        "onesd": np.ones((128, 64), np.float32),
    }


def kernel(tokens, qkv_w, qkv_b, proj_w, proj_b):
    from concourse.bass_utils import run_bass_kernel_spmd

    tokens = np.asarray(tokens, dtype=np.float32)
    qkv_w = np.asarray(qkv_w, dtype=np.float32)
    qkv_b = np.asarray(qkv_b, dtype=np.float32)
    proj_w = np.asarray(proj_w, dtype=np.float32)
    proj_b = np.asarray(proj_b, dtype=np.float32)

    nc = _build_nc_cached()
    groups = [_prep_group(qkv_w, qkv_b, proj_w, g) for g in range(2)]
    in_maps = []
    for core in range(8):
        b, g = core // 2, core % 2
        m = dict(groups[g])
        m["tokT"] = np.ascontiguousarray(tokens[b].T, dtype=np.float16)
        in_maps.append(m)

    res = run_bass_kernel_spmd(nc, in_maps, core_ids=list(range(8)))
    final = np.empty((B, N, C), np.float32)
    for b in range(B):
        acc = res.results[2 * b]["outT"] + res.results[2 * b + 1]["outT"]
        final[b] = acc.T + proj_b
    return final


# revision 17
# speedup vs baseline: 1.3242x; 1.0091x over previous
"""Multi-head attention block (B=4, N=1370, C=1024, H=16) on 8 NeuronCores.

Sharding: core i -> batch i//2, head-group i%2 (8 heads = 512 channels).
Each core computes qkv^T = W_shard @ tok_b^T, per-head attention in the
transposed (S^T) layout with the softmax denominator folded into the PV
matmul as an extra ones column, then the projection partial
final^T = proj_w[:, shard].T-slice @ out^T.  Host sums the two partials
per batch and adds proj_b.

dtypes: fp16 operands for all big matmuls (1 cyc/row); fp32 PSUM
accumulation everywhere; float32r for the softmax reciprocal/broadcast.
"""

import numpy as np

B, N, C = 4, 1370, 1024
H_PER_CORE = 8
HD = 64          # head dim
CH = 512         # channels per core
NKT = 11         # 128-key tiles (10*128 + 90)
QC = [(0, 512), (512, 512), (1024, 346)]  # query free-dim chunks (bank aligned)

_NC_CACHE = {}


def _legalize_multiwait(nc, mybir):
    """This walrus build accepts only one sync wait per instruction; Tile's
    exit drain stacks one wait per live semaphore.  Hoist extras onto no-ops
    inserted just before the offending instruction."""
    for f in nc.m.functions:
        for bb in f.blocks:
            insts = bb.instructions
            i = 0
            while i < len(insts):
                inst = insts[i]
                si = inst.sync_info
                if si is not None and len(si.on_wait) > 1:
                    waits = list(si.on_wait)
                    for j, w in enumerate(waits[:-1]):
                        nop = mybir.InstNoOp(
                            name=f"{inst.name}-waitsplit-{j}", ins=[], outs=[]
                        )
                        nop.engine = inst.engine
                        nop.sync_info = mybir.SyncInfo(on_wait=[w], on_update=[])
                        insts.insert(i, nop)
                        nc.register_instruction(nop, overwrite=True)
                        i += 1
                    inst.sync_info = mybir.SyncInfo(
                        on_wait=[waits[-1]], on_update=list(si.on_update)
                    )
                i += 1


def _interleave(a, b):
    """Merge emission thunk lists roughly proportionally (a paced over b)."""
    out = []
    ia = ib = 0
    while ia < len(a) or ib < len(b):
        ra = ia / max(len(a), 1)
        rb = ib / max(len(b), 1)
        if ib >= len(b) or (ia < len(a) and ra <= rb):
            out.append(a[ia]); ia += 1
        else:
            out.append(b[ib]); ib += 1
    return out


def build_nc():
    import concourse.bass as bass
    import concourse.mybir as mybir
    import concourse.tile as tile
    from contextlib import ExitStack

    f32 = mybir.dt.float32
    f32r = mybir.dt.float32r
    f16 = mybir.dt.float16
    AF = mybir.ActivationFunctionType
    ALU = mybir.AluOpType

    nc = bass.Bass()

    tokT = nc.dram_tensor("tokT", [C, N], f16, kind="ExternalInput")
    wqkT = nc.dram_tensor("wqkT", [C, 1024], f16, kind="ExternalInput")
    qkb = nc.dram_tensor("qkb", [1024], f32, kind="ExternalInput")
    wvT = nc.dram_tensor("wvT", [C, CH], f16, kind="ExternalInput")
    vbb = nc.dram_tensor("vbb", [128, CH], f32, kind="ExternalInput")
    pwT = nc.dram_tensor("pwT", [CH, 1024], f16, kind="ExternalInput")
    onesd = nc.dram_tensor("onesd", [128, 64], f32r, kind="ExternalInput")
    outT = nc.dram_tensor("outT", [1024, N], f32, kind="ExternalOutput")

    with tile.TileContext(nc) as tc, ExitStack() as ctx:
        persist = ctx.enter_context(tc.tile_pool(name="persist", bufs=1))
        p1 = ctx.enter_context(tc.tile_pool(name="p1", bufs=1))
        expp = ctx.enter_context(tc.tile_pool(name="expp", bufs=28))
        small = ctx.enter_context(tc.tile_pool(name="small", bufs=2))
        finp = ctx.enter_context(tc.tile_pool(name="finp", bufs=6))
        ps_att = ExitStack()
        ps_s = ps_att.enter_context(tc.tile_pool(name="ps_s", bufs=2, space="PSUM"))
        ps_qkv_stack = ExitStack()
        ps_qkv = ps_qkv_stack.enter_context(
            tc.tile_pool(name="ps_qkv", bufs=2, space="PSUM"))

        ones_t = persist.tile([128, 64], f32r)
        pwT_t = persist.tile([128, 4, 1024], f16)
        qkT_t = persist.tile([128, 8, N], f16)   # q ch 0-511 (mt 0-3), k ch (mt 4-7)
        v_t = persist.tile([128, NKT, 8 * 65], f16)  # per head: 64 V cols + ones col
        outT_t = persist.tile([128, 4, N], f16)  # attention out^T per pair

        tokT_t = p1.tile([128, 8, N], f16)
        wqkT_t = p1.tile([128, 8, 1024], f16)
        wvT_t = p1.tile([128, 8, CH], f16)
        vbb_t = p1.tile([128, CH], f32)
        qkb_t = p1.tile([128, 8], f32)
        tokT_r = tokT.rearrange("(t p) n -> p t n", p=128)
        wqkT_r = wqkT.rearrange("(t p) c -> p t c", p=128)
        wvT_r = wvT.rearrange("(t p) c -> p t c", p=128)
        for kt in range(8):
            nc.sync.dma_start(tokT_t[:, kt, :], tokT_r[:, kt, :])
            nc.sync.dma_start(wqkT_t[:, kt, :], wqkT_r[:, kt, :])
        for kt in range(8):
            nc.sync.dma_start(wvT_t[:, kt, :], wvT_r[:, kt, :])
        nc.sync.dma_start(vbb_t[:], vbb[:])
        nc.sync.dma_start(qkb_t[:], qkb.rearrange("(t p) -> p t", p=128))
        nc.sync.dma_start(ones_t[:], onesd[:])
        nc.sync.dma_start(pwT_t[:], pwT.rearrange("(t p) c -> p t c", p=128))
        vt_heads = v_t[:].rearrange("p t (h x) -> p t h x", x=65)
        nc.vector.memset(vt_heads[:, :, :, 64:65], 1.0)

        # ---- emission thunks ----
        def qk_unit(mt, qo, qw):
            def emit():
                ps = ps_qkv.tile([128, 512], f32, tag="qkv")
                for kt in range(8):
                    nc.tensor.matmul(
                        ps[:, :qw],
                        wqkT_t[:, kt, mt * 128:(mt + 1) * 128],
                        tokT_t[:, kt, qo:qo + qw],
                        start=(kt == 0), stop=(kt == 7),
                    )
                nc.vector.tensor_scalar_add(
                    qkT_t[:, mt, qo:qo + qw], ps[:, :qw], qkb_t[:, mt:mt + 1]
                )
            return emit

        def v_unit(tt):
            def emit():
                tw = 128 if tt < 10 else 90
                psv = ps_qkv.tile([128, 512], f32, tag="qkv")
                for kt in range(8):
                    nc.tensor.matmul(
                        psv[:tw, :],
                        tokT_t[:, kt, tt * 128:tt * 128 + tw],
                        wvT_t[:, kt, :],
                        start=(kt == 0), stop=(kt == 7),
                    )
                nc.vector.tensor_tensor(
                    out=vt_heads[:tw, tt, :, 0:64],
                    in0=psv[:tw, :].rearrange("p (h x) -> p h x", x=64),
                    in1=vbb_t[:tw, :].rearrange("p (h x) -> p h x", x=64),
                    op=ALU.add,
                )
            return emit

        exps = {}  # (j, half) -> [expS tiles per kt]

        def s_unit(j, kt, half):
            def emit():
                kw = 128 if kt < 10 else 90
                r0, r1 = 64 * half, 64 * half + 64
                ps = ps_s.tile([128, N], f32)
                for (qo, qw) in QC:
                    nc.tensor.matmul(
                        ps[:kw, qo:qo + qw],
                        qkT_t[r0:r1, 4 + j, kt * 128:kt * 128 + kw],
                        qkT_t[r0:r1, j, qo:qo + qw],
                    )
                e = expp.tile([128, N], f16)
                nc.scalar.activation(e[:kw, :], ps[:kw, :], AF.Exp)
                exps.setdefault((j, half), []).append(e)
            return emit

        def pv_unit(j, half, qo, qw):
            def emit():
                h = 2 * j + half
                elist = exps[(j, half)]
                pv = ps_pv.tile([65, 512], f32)
                for kt in range(NKT):
                    kw = 128 if kt < 10 else 90
                    nc.tensor.matmul(
                        pv[:, :qw],
                        v_t[:kw, kt, h * 65:(h + 1) * 65],
                        elist[kt][:kw, qo:qo + qw],
                        start=(kt == 0), stop=(kt == NKT - 1),
                    )
                rec = small.tile([65, 512], f32r, tag="rec")
                with nc.allow_low_precision(reason="f32r recip rounding"):
                    nc.vector.reciprocal(rec[64:65, :qw], pv[64:65, :qw])
                raw = small.tile([64, 512], f32, tag="raw")
                nc.vector.tensor_copy(raw[:, :qw], pv[0:64, :qw])
                bc = ps_bc.tile([64, 512], f32)
                nc.tensor.matmul(bc[:, :qw], ones_t[64:65, :], rec[64:65, :qw])
                if half == 0:
                    nc.vector.tensor_tensor(
                        out=outT_t[0:64, j, qo:qo + qw],
                        in0=bc[:, :qw], in1=raw[:, :qw], op=ALU.mult,
                    )
                else:
                    tmpb = small.tile([64, 512], f16, tag="tmpb")
                    nc.vector.tensor_tensor(
                        out=tmpb[:, :qw],
                        in0=bc[:, :qw], in1=raw[:, :qw], op=ALU.mult,
                    )
                    nc.sync.dma_start(outT_t[64:128, j, qo:qo + qw], tmpb[:, :qw])
            return emit

        def s_units(j):
            return [s_unit(j, kt, half) for kt in range(NKT) for half in (0, 1)]

        def pv_units(j):
            return [pv_unit(j, half, qo, qw) for (qo, qw) in QC for half in (0, 1)]

        # ---- emission schedule ----
        # qk tiles 0 and 4 first (pair 0 operands), then S(0) interleaved with
        # the rest of phase 1; then S(j+1) interleaved with PV(j).
        for mt in (0, 4):
            for (qo, qw) in QC:
                qk_unit(mt, qo, qw)()
        rest = [qk_unit(mt, qo, qw) for mt in (1, 5, 2, 6, 3, 7) for (qo, qw) in QC]
        rest += [v_unit(tt) for tt in range(NKT)]
        for th in _interleave(s_units(0), rest):
            th()
        ps_qkv_stack.close()
        ps_pv = ps_att.enter_context(tc.tile_pool(name="ps_pv", bufs=1, space="PSUM"))
        ps_bc = ps_att.enter_context(tc.tile_pool(name="ps_bc", bufs=1, space="PSUM"))
        for j in range(1, 4):
            for th in _interleave(s_units(j), pv_units(j - 1)):
                th()
        for th in pv_units(3):
            th()

        ps_att.close()
        ps_pj = ctx.enter_context(tc.tile_pool(name="ps_pj", bufs=4, space="PSUM"))

        # ---- projection partial ----
        for i, (qo, qw) in enumerate(QC):
            for ct in range(8):
                pp = ps_pj.tile([128, 512], f32)
                for j in range(4):
                    nc.tensor.matmul(
                        pp[:, :qw],
                        pwT_t[:, j, ct * 128:(ct + 1) * 128],
                        outT_t[:, j, qo:qo + qw],
                        start=(j == 0), stop=(j == 3),
                    )
                fin = finp.tile([128, 512], f32)
                if (ct * 3 + i) % 2 == 0:
                    nc.vector.tensor_copy(fin[:, :qw], pp[:, :qw])
                else:
                    nc.scalar.copy(fin[:, :qw], pp[:, :qw])
                nc.sync.dma_start(
                    outT[ct * 128:(ct + 1) * 128, qo:qo + qw], fin[:, :qw]
                )

    _legalize_multiwait(nc, mybir)
    return nc


def _build_nc_cached():
    if "nc" not in _NC_CACHE:
        _NC_CACHE["nc"] = build_nc()
    return _NC_CACHE["nc"]


def _prep_group(qkv_w, qkv_b, proj_w, g):
    scale = np.float32(HD ** -0.5)
    qs = slice(CH * g, CH * g + CH)
    ks = slice(1024 + CH * g, 1024 + CH * g + CH)
    vs = slice(2048 + CH * g, 2048 + CH * g + CH)
    wqk = np.concatenate([qkv_w[qs] * scale, qkv_w[ks]], axis=0)
    return {
        "wqkT": np.ascontiguousarray(wqk.T, dtype=np.float16),
        "qkb": np.concatenate([qkv_b[qs] * scale, qkv_b[ks]]).astype(np.float32),
        "wvT": np.ascontiguousarray(qkv_w[vs].T, dtype=np.float16),
        "vbb": np.ascontiguousarray(
            np.broadcast_to(qkv_b[vs], (128, CH)), dtype=np.float32
        ),
        "pwT": np.ascontiguousarray(proj_w[:, CH * g:CH * g + CH].T, dtype=np.float16),
        "onesd": np.ones((128, 64), np.float32),
    }


def kernel(tokens, qkv_w, qkv_b, proj_w, proj_b):
    from concourse.bass_utils import run_bass_kernel_spmd

    tokens = np.asarray(tokens, dtype=np.float32)
    qkv_w = np.asarray(qkv_w, dtype=np.float32)
    qkv_b = np.asarray(qkv_b, dtype=np.float32)
    proj_w = np.asarray(proj_w, dtype=np.float32)
    proj_b = np.asarray(proj_b, dtype=np.float32)

    nc = _build_nc_cached()
    groups = [_prep_group(qkv_w, qkv_b, proj_w, g) for g in range(2)]
    in_maps = []
    for core in range(8):
        b, g = core // 2, core % 2
        m = dict(groups[g])
        m["tokT"] = np.ascontiguousarray(tokens[b].T, dtype=np.float16)
        in_maps.append(m)

    res = run_bass_kernel_spmd(nc, in_maps, core_ids=list(range(8)))
    final = np.empty((B, N, C), np.float32)
    for b in range(B):
        acc = res.results[2 * b]["outT"] + res.results[2 * b + 1]["outT"]
        final[b] = acc.T + proj_b
    return final


# revision 18
# speedup vs baseline: 1.3415x; 1.0130x over previous
"""Multi-head attention block (B=4, N=1370, C=1024, H=16) on 8 NeuronCores.

Sharding: core i -> batch i//2, head-group i%2 (8 heads = 512 channels).
Each core computes qkv^T = W_shard @ tok_b^T, per-head attention in the
transposed (S^T) layout with the softmax denominator folded into the PV
matmul as an extra ones column, then the projection partial
final^T = proj_w[:, shard].T-slice @ out^T.  Host sums the two partials
per batch and adds proj_b.

dtypes: fp16 operands for all big matmuls (1 cyc/row); fp32 PSUM
accumulation everywhere; float32r for the softmax reciprocal/broadcast.
"""

import numpy as np

B, N, C = 4, 1370, 1024
H_PER_CORE = 8
HD = 64          # head dim
CH = 512         # channels per core
NKT = 11         # 128-key tiles (10*128 + 90)
QC = [(0, 512), (512, 512), (1024, 346)]  # query free-dim chunks (bank aligned)

_NC_CACHE = {}


def _legalize_multiwait(nc, mybir):
    """This walrus build accepts only one sync wait per instruction; Tile's
    exit drain stacks one wait per live semaphore.  Hoist extras onto no-ops
    inserted just before the offending instruction."""
    for f in nc.m.functions:
        for bb in f.blocks:
            insts = bb.instructions
            i = 0
            while i < len(insts):
                inst = insts[i]
                si = inst.sync_info
                if si is not None and len(si.on_wait) > 1:
                    waits = list(si.on_wait)
                    for j, w in enumerate(waits[:-1]):
                        nop = mybir.InstNoOp(
                            name=f"{inst.name}-waitsplit-{j}", ins=[], outs=[]
                        )
                        nop.engine = inst.engine
                        nop.sync_info = mybir.SyncInfo(on_wait=[w], on_update=[])
                        insts.insert(i, nop)
                        nc.register_instruction(nop, overwrite=True)
                        i += 1
                    inst.sync_info = mybir.SyncInfo(
                        on_wait=[waits[-1]], on_update=list(si.on_update)
                    )
                i += 1


def _interleave(a, b):
    """Merge emission thunk lists roughly proportionally (a paced over b)."""
    out = []
    ia = ib = 0
    while ia < len(a) or ib < len(b):
        ra = ia / max(len(a), 1)
        rb = ib / max(len(b), 1)
        if ib >= len(b) or (ia < len(a) and ra <= rb):
            out.append(a[ia]); ia += 1
        else:
            out.append(b[ib]); ib += 1
    return out


def build_nc():
    import concourse.bass as bass
    import concourse.mybir as mybir
    import concourse.tile as tile
    from contextlib import ExitStack

    f32 = mybir.dt.float32
    f32r = mybir.dt.float32r
    f16 = mybir.dt.float16
    AF = mybir.ActivationFunctionType
    ALU = mybir.AluOpType

    nc = bass.Bass()

    tokT = nc.dram_tensor("tokT", [C, N], f16, kind="ExternalInput")
    wqkT = nc.dram_tensor("wqkT", [C, 1024], f16, kind="ExternalInput")
    qkb = nc.dram_tensor("qkb", [1024], f32, kind="ExternalInput")
    wvT = nc.dram_tensor("wvT", [C, CH], f16, kind="ExternalInput")
    vbb = nc.dram_tensor("vbb", [128, CH], f32, kind="ExternalInput")
    pwT = nc.dram_tensor("pwT", [CH, 1024], f16, kind="ExternalInput")
    onesd = nc.dram_tensor("onesd", [128, 64], f32r, kind="ExternalInput")
    outT = nc.dram_tensor("outT", [1024, N], f32, kind="ExternalOutput")

    with tile.TileContext(nc) as tc, ExitStack() as ctx:
        persist = ctx.enter_context(tc.tile_pool(name="persist", bufs=1))
        p1 = ctx.enter_context(tc.tile_pool(name="p1", bufs=1))
        expp = ctx.enter_context(tc.tile_pool(name="expp", bufs=30))
        small = ctx.enter_context(tc.tile_pool(name="small", bufs=2))
        finp = ctx.enter_context(tc.tile_pool(name="finp", bufs=6))
        ps_att = ExitStack()
        ps_s = ps_att.enter_context(tc.tile_pool(name="ps_s", bufs=2, space="PSUM"))
        ps_qkv_stack = ExitStack()
        ps_qkv = ps_qkv_stack.enter_context(
            tc.tile_pool(name="ps_qkv", bufs=2, space="PSUM"))

        ones_t = persist.tile([128, 64], f32r)
        pwT_t = persist.tile([128, 4, 1024], f16)
        qkT_t = persist.tile([128, 8, N], f16)   # q ch 0-511 (mt 0-3), k ch (mt 4-7)
        v_t = persist.tile([128, NKT, 8 * 65], f16)  # per head: 64 V cols + ones col
        outT_t = persist.tile([128, 4, N], f16)  # attention out^T per pair

        tokT_t = p1.tile([128, 8, N], f16)
        wqkT_t = p1.tile([128, 8, 1024], f16)
        wvT_t = p1.tile([128, 8, CH], f16)
        vbb_t = p1.tile([128, CH], f32)
        qkb_t = p1.tile([128, 8], f32)
        tokT_r = tokT.rearrange("(t p) n -> p t n", p=128)
        wqkT_r = wqkT.rearrange("(t p) c -> p t c", p=128)
        wvT_r = wvT.rearrange("(t p) c -> p t c", p=128)
        for kt in range(8):
            nc.sync.dma_start(tokT_t[:, kt, :], tokT_r[:, kt, :])
            nc.sync.dma_start(wqkT_t[:, kt, :], wqkT_r[:, kt, :])
        for kt in range(8):
            nc.sync.dma_start(wvT_t[:, kt, :], wvT_r[:, kt, :])
        nc.sync.dma_start(vbb_t[:], vbb[:])
        nc.sync.dma_start(qkb_t[:], qkb.rearrange("(t p) -> p t", p=128))
        nc.sync.dma_start(ones_t[:], onesd[:])
        nc.sync.dma_start(pwT_t[:], pwT.rearrange("(t p) c -> p t c", p=128))
        vt_heads = v_t[:].rearrange("p t (h x) -> p t h x", x=65)
        nc.vector.memset(vt_heads[:, :, :, 64:65], 1.0)

        # ---- emission thunks ----
        def qk_unit(mt, qo, qw):
            def emit():
                ps = ps_qkv.tile([128, 512], f32, tag="qkv")
                for kt in range(8):
                    nc.tensor.matmul(
                        ps[:, :qw],
                        wqkT_t[:, kt, mt * 128:(mt + 1) * 128],
                        tokT_t[:, kt, qo:qo + qw],
                        start=(kt == 0), stop=(kt == 7),
                    )
                nc.vector.tensor_scalar_add(
                    qkT_t[:, mt, qo:qo + qw], ps[:, :qw], qkb_t[:, mt:mt + 1]
                )
            return emit

        def v_unit(tt):
            def emit():
                tw = 128 if tt < 10 else 90
                psv = ps_qkv.tile([128, 512], f32, tag="qkv")
                for kt in range(8):
                    nc.tensor.matmul(
                        psv[:tw, :],
                        tokT_t[:, kt, tt * 128:tt * 128 + tw],
                        wvT_t[:, kt, :],
                        start=(kt == 0), stop=(kt == 7),
                    )
                nc.vector.tensor_tensor(
                    out=vt_heads[:tw, tt, :, 0:64],
                    in0=psv[:tw, :].rearrange("p (h x) -> p h x", x=64),
                    in1=vbb_t[:tw, :].rearrange("p (h x) -> p h x", x=64),
                    op=ALU.add,
                )
            return emit

        exps = {}  # (j, half) -> [expS tiles per kt]

        def s_unit(j, kt, half):
            def emit():
                kw = 128 if kt < 10 else 90
                r0, r1 = 64 * half, 64 * half + 64
                ps = ps_s.tile([128, N], f32)
                for (qo, qw) in QC:
                    nc.tensor.matmul(
                        ps[:kw, qo:qo + qw],
                        qkT_t[r0:r1, 4 + j, kt * 128:kt * 128 + kw],
                        qkT_t[r0:r1, j, qo:qo + qw],
                    )
                e = expp.tile([128, N], f16)
                nc.scalar.activation(e[:kw, :], ps[:kw, :], AF.Exp)
                exps.setdefault((j, half), []).append(e)
            return emit

        def pv_unit(j, half, qo, qw):
            def emit():
                h = 2 * j + half
                elist = exps[(j, half)]
                pv = ps_pv.tile([65, 512], f32)
                for kt in range(NKT):
                    kw = 128 if kt < 10 else 90
                    nc.tensor.matmul(
                        pv[:, :qw],
                        v_t[:kw, kt, h * 65:(h + 1) * 65],
                        elist[kt][:kw, qo:qo + qw],
                        start=(kt == 0), stop=(kt == NKT - 1),
                    )
                rec = small.tile([65, 512], f32r, tag="rec")
                with nc.allow_low_precision(reason="f32r recip rounding"):
                    nc.vector.reciprocal(rec[64:65, :qw], pv[64:65, :qw])
                raw = small.tile([64, 512], f32, tag="raw")
                nc.vector.tensor_copy(raw[:, :qw], pv[0:64, :qw])
                bc = ps_bc.tile([64, 512], f32)
                nc.tensor.matmul(bc[:, :qw], ones_t[64:65, :], rec[64:65, :qw])
                if half == 0:
                    nc.vector.tensor_tensor(
                        out=outT_t[0:64, j, qo:qo + qw],
                        in0=bc[:, :qw], in1=raw[:, :qw], op=ALU.mult,
                    )
                else:
                    tmpb = small.tile([64, 512], f16, tag="tmpb")
                    nc.vector.tensor_tensor(
                        out=tmpb[:, :qw],
                        in0=bc[:, :qw], in1=raw[:, :qw], op=ALU.mult,
                    )
                    nc.sync.dma_start(outT_t[64:128, j, qo:qo + qw], tmpb[:, :qw])
            return emit

        def s_units(j):
            return [s_unit(j, kt, half) for kt in range(NKT) for half in (0, 1)]

        def pv_units(j):
            return [pv_unit(j, half, qo, qw) for (qo, qw) in QC for half in (0, 1)]

        # ---- emission schedule ----
        # qk tiles 0 and 4 first (pair 0 operands), then S(0) interleaved with
        # the rest of phase 1; then S(j+1) interleaved with PV(j).
        for mt in (0, 4):
            for (qo, qw) in QC:
                qk_unit(mt, qo, qw)()
        rest = [qk_unit(mt, qo, qw) for mt in (1, 5, 2, 6, 3, 7) for (qo, qw) in QC]
        rest += [v_unit(tt) for tt in range(NKT)]
        for th in _interleave(s_units(0), rest):
            th()
        ps_qkv_stack.close()
        ps_pv = ps_att.enter_context(tc.tile_pool(name="ps_pv", bufs=1, space="PSUM"))
        ps_bc = ps_att.enter_context(tc.tile_pool(name="ps_bc", bufs=1, space="PSUM"))
        for j in range(1, 4):
            for th in _interleave(s_units(j), pv_units(j - 1)):
                th()
        for th in pv_units(3):
            th()

        ps_att.close()
        ps_pj = ctx.enter_context(tc.tile_pool(name="ps_pj", bufs=4, space="PSUM"))

        # ---- projection partial ----
        for i, (qo, qw) in enumerate(QC):
            for ct in range(8):
                pp = ps_pj.tile([128, 512], f32)
                for j in range(4):
                    nc.tensor.matmul(
                        pp[:, :qw],
                        pwT_t[:, j, ct * 128:(ct + 1) * 128],
                        outT_t[:, j, qo:qo + qw],
                        start=(j == 0), stop=(j == 3),
                    )
                fin = finp.tile([128, 512], f32)
                if (ct * 3 + i) % 2 == 0:
                    nc.vector.tensor_copy(fin[:, :qw], pp[:, :qw])
                else:
                    nc.scalar.copy(fin[:, :qw], pp[:, :qw])
                nc.sync.dma_start(
                    outT[ct * 128:(ct + 1) * 128, qo:qo + qw], fin[:, :qw]
                )

    _legalize_multiwait(nc, mybir)
    return nc


def _build_nc_cached():
    if "nc" not in _NC_CACHE:
        _NC_CACHE["nc"] = build_nc()
    return _NC_CACHE["nc"]


def _prep_group(qkv_w, qkv_b, proj_w, g):
    scale = np.float32(HD ** -0.5)
    qs = slice(CH * g, CH * g + CH)
    ks = slice(1024 + CH * g, 1024 + CH * g + CH)
    vs = slice(2048 + CH * g, 2048 + CH * g + CH)
    wqk = np.concatenate([qkv_w[qs] * scale, qkv_w[ks]], axis=0)
    return {
        "wqkT": np.ascontiguousarray(wqk.T, dtype=np.float16),
        "qkb": np.concatenate([qkv_b[qs] * scale, qkv_b[ks]]).astype(np.float32),
        "wvT": np.ascontiguousarray(qkv_w[vs].T, dtype=np.float16),
        "vbb": np.ascontiguousarray(
            np.broadcast_to(qkv_b[vs], (128, CH)), dtype=np.float32
        ),
        "pwT": np.ascontiguousarray(proj_w[:, CH * g:CH * g + CH].T, dtype=np.float16),
        "onesd": np.ones((128, 64), np.float32),
    }


def kernel(tokens, qkv_w, qkv_b, proj_w, proj_b):
    from concourse.bass_utils import run_bass_kernel_spmd

    tokens = np.asarray(tokens, dtype=np.float32)
    qkv_w = np.asarray(qkv_w, dtype=np.float32)
    qkv_b = np.asarray(qkv_b, dtype=np.float32)
    proj_w = np.asarray(proj_w, dtype=np.float32)
    proj_b = np.asarray(proj_b, dtype=np.float32)

    nc = _build_nc_cached()
    groups = [_prep_group(qkv_w, qkv_b, proj_w, g) for g in range(2)]
    in_maps = []
    for core in range(8):
        b, g = core // 2, core % 2
        m = dict(groups[g])
        m["tokT"] = np.ascontiguousarray(tokens[b].T, dtype=np.float16)
        in_maps.append(m)

    res = run_bass_kernel_spmd(nc, in_maps, core_ids=list(range(8)))
    final = np.empty((B, N, C), np.float32)
    for b in range(B):
        acc = res.results[2 * b]["outT"] + res.results[2 * b + 1]["outT"]
        final[b] = acc.T + proj_b
    return final
